# revision 1
# baseline (speedup 1.0000x reference)
"""Trainium2 Bass kernel for a 3-layer GraphSAGE-GCN (gnn_message_passing).

Math (per layer, commuting the dense matmul through the linear aggregation):
    Y_l   = h_{l-1} @ W_l^T                      (dense per-node matmul)
    h_l   = relu(inv ⊙ (A + I) Y_l)              (edge gather + scatter-add)
with inv = 1/(deg_in + 1) (self-loops appended so (A+I) = plain edge sum).

Distribution: destination nodes (and their incoming edges) are sharded
across 8 NeuronCores; the small [6250, D] per-core matmul outputs are
AllGather'ed into a full [50000, D] gather source between layers.

Scatter-add on device: edges are host-bucketed by (dst block of 128,
src half); per bucket a dma_gather pulls Y[src] rows into SBUF
[128 edges x D], a one-hot selection matrix O[p, j] = (dst_slot[p] == j)
is built with one tensor_scalar(is_equal) against a row-iota constant,
and PE matmul O^T @ G accumulates the per-dst-block aggregate in PSUM.
The src half split exists because dma_gather indices are int16.
"""

import math

import numpy as np

import concourse.bacc as bacc
import concourse.bass as bass
import concourse.mybir as mybir
import concourse.tile as tile
from concourse.bass_utils import run_bass_kernel_spmd
from concourse.masks import make_identity

# Problem constants (hardcoded per harness contract).
N = 50000
DIN = 128
DH = 128
DZ = 64
NCORES = 8
P = 128
NLOC = N // NCORES           # 6250 destination nodes per core
NBLK = (NLOC + P - 1) // P   # 49 dst blocks per core
HALF = 25000                 # src index split so gather indices fit int16

F32 = mybir.dt.float32
BF16 = mybir.dt.float16
I16 = mybir.dt.int16
GATHER_BF16 = True
DEBUG_TAPS = False
DEBUG_TAP = (0, 0)
BUILD_STAGES = 6
NQUEUES = 4
CALL_CHUNKS = 8
GBUFS = 12
PREFETCH_GATHERS = True
PF_DIST = 3
OH_DIST = 1
OPOOL_BUFS = 3
HPOOL_BUFS = 6
AGG_BUFS = 4
MOCK_COLLECTIVES = False
SKIP_GATHER = False
SKIP_MM = False
LIMIT_BLOCKS = None
REPEATS = 1


def _cdiv(a, b):
    return (a + b - 1) // b


def _preprocess(x, edge_index):
    """Host-side graph partitioning: bucket edges by (core, dst block, src
    half), pad each bucket to a fixed chunk budget, build the int16 gather
    index arrays, the one-hot dst-slot arrays, degree counts and transposed
    node features."""
    src = np.asarray(edge_index[0], dtype=np.int64)
    dst = np.asarray(edge_index[1], dtype=np.int64)
    # self term handled in the epilogue (adds y_loc block), not as edges:
    # keeps the lo/hi halves balanced across cores and saves ~10% chunks
    cnt = np.bincount(dst, minlength=N).astype(np.float32) + 1.0  # deg + 1

    core = dst // NLOC
    ldst = dst - core * NLOC
    blk = ldst // P
    slot = ldst % P
    half = (src >= HALF).astype(np.int64)
    ncells = NCORES * NBLK * 2
    cell = (core * 2 + half) * NBLK + blk  # half-major: lo cells, then hi

    order = np.argsort(cell, kind="stable")
    cell_s = cell[order]
    half_s = half[order]
    vals = (src[order] - half_s * HALF).astype(np.int16)
    slot_s = slot[order].astype(np.float32)

    counts = np.bincount(cell_s, minlength=ncells).astype(np.int64)
    by_half = counts.reshape(NCORES, 2, NBLK)
    C_LO = max(1, _cdiv(int(by_half[:, 0, :].max()), P))
    C_HI = max(1, _cdiv(int(by_half[:, 1, :].max()), P))
    CTOT = C_LO + C_HI
    CPC = NBLK * CTOT          # chunks per core
    COLS = CPC * 8             # int16 idx columns per core

    cell_start = np.zeros(ncells + 1, np.int64)
    cell_start[1:] = np.cumsum(counts)
    q = np.arange(len(cell_s)) - cell_start[cell_s]  # rank within own cell

    core_s = cell_s // (NBLK * 2)
    b_s = cell_s % NBLK
    # idx stream layout per core is half-major (lo chunks then hi chunks) so
    # packed gather calls read contiguous index columns; the dmod (one-hot)
    # layout is block-major so one broadcast tensor_tensor builds a whole
    # block's selection matrices.
    chunk0_i = np.where(half_s == 0, b_s * C_LO, NBLK * C_LO + b_s * C_HI)
    chunk0_d = b_s * CTOT + half_s * C_LO

    # Pad positions gather row 0 (finite data, killed by the all-(-1)
    # one-hot column); this keeps every gather-tile row defined.
    idx_arr = np.zeros((NCORES, 16, COLS), np.int16)
    idx_arr[core_s, q % 16, chunk0_i * 8 + q // 16] = vals
    mod_arr = np.full((NCORES, P, CPC), -1.0, np.float16)
    mod_arr[core_s, q % P, chunk0_d + q // P] = slot_s

    cnt_arr = np.ones((NCORES, NBLK * P), np.float32)
    cnt_arr[:, :NLOC] = cnt.reshape(NCORES, NLOC)
    cnt_arr = np.ascontiguousarray(
        cnt_arr.reshape(NCORES, NBLK, P).transpose(0, 2, 1))  # [NC, P, NBLK]

    x = np.asarray(x, dtype=np.float32)
    x_pad = np.zeros((NCORES, NBLK * P, DIN), np.float32)
    x_pad[:, :NLOC] = x.reshape(NCORES, NLOC, DIN)
    xT = np.ascontiguousarray(x_pad.transpose(0, 2, 1))  # [NC, DIN, NBLK*P]

    iota = np.tile(np.arange(P, dtype=np.float16), (P, 1))
    iota_big = np.tile(iota, (1, CTOT))

    return dict(C_LO=C_LO, C_HI=C_HI, idx=idx_arr, dmod=mod_arr, cnt=cnt_arr,
                xT=xT, iota=iota_big)


def _build(C_LO, C_HI):
    """Build + compile the SPMD Bass program (identical on all cores)."""
    CTOT = C_LO + C_HI
    CPC = NBLK * CTOT
    COLS = CPC * 8

    nc = bacc.Bacc("TRN2", target_bir_lowering=False, debug=False,
                   num_devices=NCORES, num_swdge_queues=NQUEUES)

    xT_d = nc.dram_tensor("xT", [DIN, NBLK * P], F32, kind="ExternalInput")
    idx_d = nc.dram_tensor("idx", [16, COLS], I16, kind="ExternalInput")
    dmod_d = nc.dram_tensor("dmod", [P, CPC], BF16, kind="ExternalInput")
    cnt_d = nc.dram_tensor("cnt", [P, NBLK], F32, kind="ExternalInput")
    iota_d = nc.dram_tensor("iota", [P, CTOT * P], BF16,
                           kind="ExternalInput")
    w1t_d = nc.dram_tensor("w1t", [DIN, DH], F32, kind="ExternalInput")
    w2t_d = nc.dram_tensor("w2t", [DH, DH], F32, kind="ExternalInput")
    w3t_d = nc.dram_tensor("w3t", [DH, DZ], F32, kind="ExternalInput")
    out_d = nc.dram_tensor("out", [NLOC, DZ], F32, kind="ExternalOutput")

    if DEBUG_TAPS:
        glo_dbg = nc.dram_tensor("glodbg", [P, C_LO * DH], F32,
                                 kind="ExternalOutput")
        ghi_dbg = nc.dram_tensor("ghidbg", [P, C_HI * DH], F32,
                                 kind="ExternalOutput")
        oh_dbg = nc.dram_tensor("ohdbg", [P, P], F32, kind="ExternalOutput")
        h_dbg = nc.dram_tensor("hdbg", [P, DH], F32, kind="ExternalOutput")

    # Y3 rows are padded to 128 fp16 (256B, the dma_gather minimum elem);
    # cols 64:128 are never written or read -- same gather bytes as 64xf32,
    # but the one-hot and matmuls take the fp16 fast paths.
    ydt = [BF16, BF16, BF16]
    y_loc = [nc.dram_tensor(f"y{l}loc", [NLOC, d], ydt[l - 1])
             for l, d in ((1, DH), (2, DH), (3, DH))]
    y_full = [nc.dram_tensor(f"y{l}full", [N, d], ydt[l - 1],
                             addr_space="Shared")
              for l, d in ((1, DH), (2, DH), (3, DH))]

    rows_of = [min(P, NLOC - b * P) for b in range(NBLK)]

    with tile.TileContext(nc) as tc:
        with (
            tc.tile_pool(name="pers", bufs=1) as pers,
            tc.tile_pool(name="gpool", bufs=GBUFS) as gpool,
            tc.tile_pool(name="opool", bufs=OPOOL_BUFS) as opool,
            tc.tile_pool(name="hpool", bufs=HPOOL_BUFS) as hpool,
            tc.tile_pool(name="agg_ps", bufs=AGG_BUFS, space="PSUM") as agg_pp,
            tc.tile_pool(name="tr_ps", bufs=8 - AGG_BUFS - 1, space="PSUM") as tr_pp,
            tc.tile_pool(name="y_ps", bufs=1, space="PSUM") as y_pp,
        ):
            # --- persistent tiles -------------------------------------------
            # The Q7 descriptor generators read the index stream through
            # their own 16-partition groups: cpu0 (rx) reads partitions
            # 0-15, cpu1 (tx) reads 16-31 -- the indices must be replicated
            # into both groups.
            idx_sb = pers.tile([P, COLS], I16)
            for gidx in range(8):
                nc.sync.dma_start(idx_sb[gidx * 16:(gidx + 1) * 16, :],
                                  idx_d[:, :])
            dmod_sb = pers.tile([P, CPC], BF16)
            nc.sync.dma_start(dmod_sb[:], dmod_d[:, :])
            iota_sb = pers.tile([P, CTOT * P], BF16)
            nc.sync.dma_start(iota_sb[:], iota_d[:, :])
            cnt_sb = pers.tile([P, NBLK], F32)
            nc.sync.dma_start(cnt_sb[:], cnt_d[:, :])
            inv_sb = pers.tile([P, NBLK], F32)
            nc.vector.reciprocal(inv_sb[:], cnt_sb[:])
            w1t_sb = pers.tile([DIN, DH], F32)
            nc.sync.dma_start(w1t_sb[:], w1t_d[:, :])
            w2t_sb = pers.tile([DH, DH], F32)
            nc.sync.dma_start(w2t_sb[:], w2t_d[:, :])
            w3t_sb = pers.tile([DH, DZ], F32)
            nc.sync.dma_start(w3t_sb[:], w3t_d[:, :])
            ident = pers.tile([P, P], F32)
            make_identity(nc, ident[:])
            xT_sb = pers.tile([DIN, NBLK * P], F32)
            nc.sync.dma_start(xT_sb[:], xT_d[:, :])


            # --- prologue: Y1 = x @ W1^T ------------------------------------
            for b in range(NBLK):
                y_ps = y_pp.tile([P, DH], F32, tag="yps")
                nc.tensor.matmul(y_ps[:], lhsT=xT_sb[:, b * P:(b + 1) * P],
                                 rhs=w1t_sb[:], start=True, stop=True)
                y_sb = hpool.tile([P, DH], ydt[0], tag="ysb")
                nc.vector.tensor_copy(y_sb[:], y_ps[:])
                r = rows_of[b]
                nc.sync.dma_start(y_loc[0][b * P:b * P + r, :], y_sb[:r, :])

            def allgather(li):
                if MOCK_COLLECTIVES:
                    # timing-only single-core variant (TimelineSim has no
                    # collectives); data correctness not preserved
                    nc.sync.dma_start(y_full[li][0:NLOC, :], y_loc[li][:, :])
                    return
                nc.gpsimd.collective_compute(
                    "AllGather", mybir.AluOpType.bypass,
                    ins=[y_loc[li][:, :]], outs=[y_full[li][:, :]],
                    replica_groups=[list(range(NCORES))])

            call_no = [0]

            def agg_layer(li, D, w_next_sb, D_next):
                EL = DH  # gather elem width (Y3 rows are padded to DH)
                gdt = ydt[li]
                """Aggregate y_full[li] into h, then either compute the next
                layer's Y (w_next_sb) or write the final output.

                Gathers are packed into calls of up to CALL_C chunks
                (descriptor-ring capacity limits one dma_gather to ~1024
                indices); calls cross dst-block boundaries.
                """
                yf = y_full[li]
                src_ap = [yf[0:HALF, :], yf[HALF:N, :]]
                n_ch = [NBLK * C_LO, NBLK * C_HI]   # chunks per half stream
                col0 = [0, NBLK * C_LO * 8]         # idx col base per half
                tiles = [[], []]                    # emitted gather tiles
                CALL_C = CALL_CHUNKS

                def ensure_call(h, o):
                    # Emit gather calls for half h until chunk ordinal o is
                    # covered; returns (tile, slice offset within tile).
                    k = o // CALL_C
                    while len(tiles[h]) <= k:
                        kk = len(tiles[h])
                        nch = min(CALL_C, n_ch[h] - kk * CALL_C)
                        g = gpool.tile([P, nch * EL], gdt, tag=f"g{h}")
                        c0 = col0[h] + kk * CALL_C * 8
                        if not SKIP_GATHER:
                            nc.gpsimd.dma_gather(
                                g[:].rearrange("p (c d) -> p c d", d=EL),
                                src_ap[h], idx_sb[:, c0:c0 + nch * 8],
                                nch * P, nch * P, EL,
                                queue_num=call_no[0] % NQUEUES)
                        else:
                            nc.vector.memset(g[:, :P], 0)
                        call_no[0] += 1
                        tiles[h].append(g)
                    return tiles[h][k], (o % CALL_C)

                nblk = NBLK if LIMIT_BLOCKS is None else LIMIT_BLOCKS

                def emit_oh(b):
                    # one broadcast is_equal builds all CTOT selection
                    # matrices of block b: oh[p, k, j] =
                    # (iota[j] == dmod[p, b*CTOT+k])
                    oh = opool.tile([P, CTOT * P], gdt, tag="oh")
                    dm = dmod_sb[:, b * CTOT:(b + 1) * CTOT]
                    nc.vector.tensor_tensor(
                        out=oh[:].rearrange("p (k j) -> p k j", j=P),
                        in0=iota_sb[:].rearrange("p (k j) -> p k j", j=P),
                        in1=dm.to_broadcast([P, CTOT, P]),
                        op=mybir.AluOpType.is_equal)
                    return oh

                def emit_tail(b, agg):
                    r = rows_of[b]
                    ys = hpool.tile([P, D], ydt[li], tag="yself")
                    nc.sync.dma_start(ys[:r, :],
                                      y_loc[li][b * P:b * P + r, 0:D])
                    t_sb = hpool.tile([P, D], F32, tag="tsb")
                    nc.vector.tensor_tensor(out=t_sb[:], in0=agg[:],
                                            in1=ys[:],
                                            op=mybir.AluOpType.add)
                    h_sb = hpool.tile([P, D], F32, tag="hsb")
                    nc.scalar.activation(
                        h_sb[:], t_sb[:], mybir.ActivationFunctionType.Relu,
                        scale=inv_sb[:, b:b + 1])
                    if w_next_sb is None:
                        nc.sync.dma_start(out_d[b * P:b * P + r, :],
                                          h_sb[:r, :])
                    else:
                        hT_ps = tr_pp.tile([P, P], F32, tag="htps")
                        nc.tensor.transpose(hT_ps[:], h_sb[:], ident[:])
                        hT_sb = hpool.tile([P, P], F32, tag="htsb")
                        nc.vector.tensor_copy(hT_sb[:], hT_ps[:])
                        y_ps = y_pp.tile([P, D_next], F32, tag="yps")
                        nc.tensor.matmul(y_ps[:], lhsT=hT_sb[:],
                                         rhs=w_next_sb[:], start=True,
                                         stop=True)
                        y_sb = hpool.tile([P, D_next], ydt[li + 1],
                                          tag="ysb")
                        nc.vector.tensor_copy(y_sb[:], y_ps[:])
                        nc.sync.dma_start(
                            y_loc[li + 1][b * P:b * P + r, 0:D_next],
                            y_sb[:r, :])

                # Software pipeline: the one-hot for block b+1 is emitted
                # before block b's matmuls (DVE computes it while PE chews on
                # block b), and block b's epilogue tail is deferred behind
                # block b+1's matmuls so per-engine FIFO order never makes
                # DVE/ACT wait on PE round trips.
                def prefetch_gathers(b2):
                    # pull block b2's gather calls into the stream early so
                    # their DMA completion latency hides under block b2-1's
                    # matmuls
                    if b2 >= nblk:
                        return
                    for h in (0, 1):
                        ch = (C_LO, C_HI)[h]
                        ensure_call(h, min((b2 + 1) * ch - 1, n_ch[h] - 1))

                oh_q = [emit_oh(i) for i in range(min(OH_DIST, nblk))]
                if PREFETCH_GATHERS:
                    prefetch_gathers(0)
                pending = None
                for b in range(nblk):
                    if b + OH_DIST < nblk:
                        oh_q.append(emit_oh(b + OH_DIST))
                    if PREFETCH_GATHERS:
                        prefetch_gathers(b + PF_DIST)
                    if pending is not None:
                        emit_tail(*pending)
                    oh_cur = oh_q.pop(0)
                    agg = agg_pp.tile([P, D], F32, tag="agg")
                    for ci in range(CTOT):
                        h = 0 if ci < C_LO else 1
                        c = ci if ci < C_LO else ci - C_LO
                        o = b * (C_LO, C_HI)[h] + c
                        g, pos = ensure_call(h, o)
                        if SKIP_MM and ci > 0:
                            continue
                        nc.tensor.matmul(
                            agg[:], lhsT=oh_cur[:, ci * P:(ci + 1) * P],
                            rhs=g[:, pos * EL:pos * EL + D],
                            start=(ci == 0),
                            stop=(ci == CTOT - 1 or SKIP_MM))
                    pending = (b, agg)
                emit_tail(*pending)

            def prologue():
                for b in range(NBLK):
                    y_ps = y_pp.tile([P, DH], F32, tag="yps")
                    nc.tensor.matmul(y_ps[:],
                                     lhsT=xT_sb[:, b * P:(b + 1) * P],
                                     rhs=w1t_sb[:], start=True, stop=True)
                    y_sb = hpool.tile([P, DH], ydt[0], tag="ysb")
                    nc.vector.tensor_copy(y_sb[:], y_ps[:])
                    r = rows_of[b]
                    nc.sync.dma_start(y_loc[0][b * P:b * P + r, :],
                                      y_sb[:r, :])

            stages = [
                lambda: allgather(0),
                lambda: agg_layer(0, DH, w2t_sb, DH),
                lambda: allgather(1),
                lambda: agg_layer(1, DH, w3t_sb, DZ),
                lambda: allgather(2),
                lambda: agg_layer(2, DZ, None, None),
            ]
            for st in stages[:BUILD_STAGES]:
                st()
            for _ in range(REPEATS - 1):
                prologue()
                for st in stages[:BUILD_STAGES]:
                    st()

    nc.compile()
    return nc


_cache = {}


def _get_nc(C_LO, C_HI):
    key = (C_LO, C_HI)
    if key not in _cache:
        _cache[key] = _build(C_LO, C_HI)
    return _cache[key]


def kernel(x, edge_index, W1, W2, W3, _trace=False):
    prep = _preprocess(x, edge_index)
    nc = _get_nc(prep["C_LO"], prep["C_HI"])

    w1t = np.ascontiguousarray(np.asarray(W1, np.float32).T)
    w2t = np.ascontiguousarray(np.asarray(W2, np.float32).T)
    w3t = np.ascontiguousarray(np.asarray(W3, np.float32).T)
    in_maps = []
    for c in range(NCORES):
        in_maps.append({
            "xT": prep["xT"][c],
            "idx": prep["idx"][c],
            "dmod": prep["dmod"][c],
            "cnt": prep["cnt"][c],
            "iota": prep["iota"],
            "w1t": w1t, "w2t": w2t, "w3t": w3t,
        })
    res = run_bass_kernel_spmd(nc, in_maps, list(range(NCORES)),
                               trace=_trace)
    out = np.concatenate([res.results[c]["out"] for c in range(NCORES)],
                         axis=0).astype(np.float32)
    if _trace:
        kernel._last_results = res
    return out

